# revision 1
# baseline (speedup 1.0000x reference)
"""Trainium2 Bass kernel for single-head causal attention (B=4, T=2048, C=2048).

Sharding: 8 cores = 4 batches x 2 t-interleave. Core (b, h) owns the 256-row
blocks {h, 2+h, 4+h, 6+h} of batch b (interleaved for causal load balance).
The two cores of a batch each compute HALF of K.T and V and exchange both in
a SINGLE 2-core AllGather (packed flat buffer), overlapped with the Q
projection. Attention runs in the "transposed domain" (scores.T = [s, t]) so
every matmul consumes naturally laid-out operands: exp(scale*s + additive
mask) without normalization, softmax denominators via ones-matmul partition
reduction, folded in as a per-partition scale on the final-projection output,
which lands in natural [t, e] layout. Host pre-transposes x / weights (part
of sharding prep) and gathers per-core outputs.

All matmuls run in bf16 (fp32 PSUM accumulation). Unlike fp32r -- whose
matmuls self-load the 128-row stationary operand every instruction (a
(128+N)-cycle cost) -- bf16 weights load via separate LDWEIGHTS with fast
weight load, double-buffered behind the previous matmul's streaming, so a
matmul costs ~N cycles. bf16 also halves DMA + collective bytes. K.T, V and
Q.T stay resident in SBUF for the whole attention phase (no DRAM round-trips
inside the k-loops). A short warm-up matmul burst at kernel start releases
the PE HAM clock-gate (1.2 -> 2.4 GHz) while the first DMAs are in flight.
"""
import sys

sys.path.insert(0, "/opt/trn_rl_repo")
import numpy as np
from ml_dtypes import bfloat16

_CACHE = {}

B = 4
T_FULL = 2048
C_FULL = 2048
NEG = -1e30


def _build(T_, C_, reps=1):
    import concourse.bacc as bacc
    import concourse.mybir as mybir
    import concourse.tile as tile

    F32 = mybir.dt.float32
    BF16 = mybir.dt.bfloat16
    AF = mybir.ActivationFunctionType
    SCALE = 1.0 / float(np.sqrt(C_FULL))

    CC = C_ // 128      # contraction 128-chunks (also d-chunks)
    NE = C_ // 512      # e-512 chunks for the final projection
    NBO = T_ // 512     # owned 256-blocks per core (j range)
    TOWN = NBO * 256    # owned rows per core
    SK = T_ // 512      # s-512 chunks
    NS = T_ // 128      # s-128 chunks
    NQ = TOWN // 512    # q 512-col chunks

    nc = bacc.Bacc("TRN2", target_bir_lowering=False, debug=False, num_devices=8)
    xTh_d = nc.declare_dram_parameter("xTh", [C_, T_ // 2], BF16, isOutput=False)
    xTq_d = nc.declare_dram_parameter("xTq", [C_, TOWN], BF16, isOutput=False)
    # weights host-pre-tiled to the exact SBUF tile layouts so the loads are
    # contiguous (4KB per partition line vs 256B strided descriptors)
    WqP_d = nc.declare_dram_parameter("WqP", [CC, 128, CC, 128], BF16, isOutput=False)
    WkP_d = nc.declare_dram_parameter("WkP", [CC, 128, CC, 128], BF16, isOutput=False)
    WvP_d = nc.declare_dram_parameter("WvP", [NE, 128, CC, 512], BF16, isOutput=False)
    WoP_d = nc.declare_dram_parameter("WoP", [NE, 128, CC, 512], BF16, isOutput=False)
    mb_d = nc.declare_dram_parameter("mb", [NBO, 4, 128, 256], BF16, isOutput=False)
    ones_d = nc.declare_dram_parameter("ones", [128, 2], BF16, isOutput=False)
    y_d = nc.declare_dram_parameter("y", [TOWN, C_], F32, isOutput=True)

    HKT = (T_ // 2) * C_     # elems in a K.T (or V) half
    SSB = HKT // 2           # elems in one 512-s-block of the K.T half

    with tile.TileContext(nc) as tc:
        with tc.tile_pool(name="dram", bufs=1, space="DRAM") as dram:
            # K.T-half ++ V-half packed flat so ONE AllGather exchanges both
            KVh = dram.tile([2 * HKT], BF16, tag="kvh")
            KVs = dram.tile([2, 2 * HKT], BF16, tag="kvs")
            # partition-major so P3's reload is one fat contiguous DMA
            OT_j = [dram.tile([128, CC, 256], BF16, tag=f"ot{j}", name=f"otj{j}") for j in range(NBO)]
            KT_h = KVh[0:HKT].rearrange("(ss d s) -> ss d s", ss=2, d=C_)
            V_h = KVh[HKT:2 * HKT].rearrange("(s d) -> s d", d=C_)

            with tc.tile_pool(name="stage", bufs=4) as stage:
                # warm the PE (HAM clock-gate releases after ~3.4us of
                # activity) while the first input DMAs are in flight
                with (
                    tc.tile_pool(name="warm", bufs=1) as pool_warm,
                    tc.tile_pool(name="pswm", bufs=1, space="PSUM") as pswm,
                ):
                    wt = pool_warm.tile([128, 512], BF16, tag="warm")
                    nc.vector.memset(wt[:], 0.0)
                    wps = pswm.tile([128, 512], F32, tag="warmps")
                    for _ in range(24):
                        nc.tensor.matmul(
                            wps[:], wt[:, 0:128], wt[:], start=True, stop=True
                        )
                for _rep in range(reps):
                    with tc.tile_pool(name="qsb", bufs=1) as pool_qsb:
                        QT_sb = pool_qsb.tile([128, CC, TOWN], BF16, tag="qtsb")
                        with (
                            tc.tile_pool(name="xtq", bufs=1) as pool_xtq,
                            tc.tile_pool(name="wq", bufs=2) as pool_wq,
                            tc.tile_pool(name="wv", bufs=2) as pool_wv,
                        ):
                            xtq = pool_xtq.tile([128, CC, TOWN], BF16, tag="xtq")
                            wq0 = pool_wq.tile([128, CC, 128], BF16, tag="wq", name="wq0")
                            wv0 = pool_wv.tile([128, CC, 512], BF16, tag="wv", name="wv0")
                            # ==== P1a: K.T = WkT.T @ xT -> KT_h [d, s] ====
                            with tc.tile_pool(name="xt", bufs=1) as pool_xt:
                                xt = pool_xt.tile([128, CC, T_ // 2], BF16, tag="xt")
                                for c in range(CC):
                                    nc.scalar.dma_start(
                                        xt[:, c, :], xTh_d[128 * c:128 * c + 128, :]
                                    )
                                with (
                                    tc.tile_pool(name="wk", bufs=2) as pool_w,
                                    tc.tile_pool(name="psk", bufs=8, space="PSUM") as psk,
                                ):
                                    for d in range(CC):
                                        wk = pool_w.tile([128, CC, 128], BF16, tag="wk")
                                        nc.scalar.dma_start(wk[:], WkP_d[d])
                                        kps = [
                                            psk.tile([128, 512], F32, tag="kps", name=f"kps{d}_{ss}")
                                            for ss in range(SK // 2)
                                        ]
                                        for c in range(CC):
                                            for ss in range(SK // 2):
                                                nc.tensor.matmul(
                                                    kps[ss][:],
                                                    wk[:, c, :],
                                                    xt[:, c, 512 * ss:512 * ss + 512],
                                                    start=(c == 0),
                                                    stop=(c == CC - 1),
                                                )
                                        for ss in range(SK // 2):
                                            st = stage.tile([128, 512], BF16, tag="st512")
                                            nc.vector.tensor_copy(st[:], kps[ss][:])
                                            nc.sync.dma_start(
                                                KT_h[ss, 128 * d:128 * d + 128, :], st[:]
                                            )
                                # P1b's first weight, then P1c's inputs
                                # (queued after all of P1a's weight loads)
                                nc.scalar.dma_start(wv0[:], WvP_d[0])
                                for c in range(CC):
                                    nc.scalar.dma_start(
                                        xtq[:, c, :], xTq_d[128 * c:128 * c + 128, :]
                                    )
                                nc.scalar.dma_start(wq0[:], WqP_d[0])
                                # ==== P1b: V = xT.T @ WvT -> V_h [s, d] ====
                                with (
                                    tc.tile_pool(name="psv", bufs=4, space="PSUM") as psv,
                                ):
                                    for dd in range(C_ // 512):
                                        if dd == 0:
                                            wv = wv0
                                        else:
                                            wv = pool_wv.tile([128, CC, 512], BF16, tag="wv")
                                            nc.scalar.dma_start(wv[:], WvP_d[dd])
                                        for s in range(NS // 2):
                                            vps = psv.tile([128, 512], F32, tag="vps")
                                            for c in range(CC):
                                                nc.tensor.matmul(
                                                    vps[:],
                                                    xt[:, c, 128 * s:128 * s + 128],
                                                    wv[:, c, :],
                                                    start=(c == 0),
                                                    stop=(c == CC - 1),
                                                )
                                            st = stage.tile([128, 512], BF16, tag="st512")
                                            nc.vector.tensor_copy(st[:], vps[:])
                                            nc.sync.dma_start(
                                                V_h[128 * s:128 * s + 128, 512 * dd:512 * dd + 512],
                                                st[:],
                                            )
                            # ==== exchange K/V halves within batch pairs ====
                            groups = [[0, 1], [2, 3], [4, 5], [6, 7]]
                            nc.gpsimd.collective_compute(
                                "AllGather",
                                mybir.AluOpType.bypass,
                                replica_groups=groups,
                                ins=[KVh[:]],
                                outs=[KVs[:]],
                            )
                            # ==== P1c: Q.T = WqT.T @ xTq -> QT_sb (SBUF) ====
                            with (
                                tc.tile_pool(name="psq", bufs=4, space="PSUM") as psq,
                            ):
                                for d in range(CC):
                                    if d == 0:
                                        wq = wq0
                                    else:
                                        wq = pool_wq.tile([128, CC, 128], BF16, tag="wq")
                                        nc.scalar.dma_start(wq[:], WqP_d[d])
                                    for tt in range(NQ):
                                        qps = psq.tile([128, 512], F32, tag="qps")
                                        for c in range(CC):
                                            nc.tensor.matmul(
                                                qps[:],
                                                wq[:, c, :],
                                                xtq[:, c, 512 * tt:512 * tt + 512],
                                                start=(c == 0),
                                                stop=(c == CC - 1),
                                            )
                                        nc.vector.tensor_copy(
                                            QT_sb[:, d, 512 * tt:512 * tt + 512], qps[:]
                                        )

                        # ======== P2: attention per owned block j ========
                        pool_cst_cm = tc.tile_pool(name="cst", bufs=1)
                        pool_cst = pool_cst_cm.__enter__()
                        onest = pool_cst.tile([128, 2], BF16, tag="ones")
                        nc.scalar.dma_start(onest[:], ones_d[:])
                        recipt = pool_cst.tile([128, 2 * NBO], F32, tag="recip")
                        with (
                            tc.tile_pool(name="kvsb", bufs=1) as pool_kvsb,
                            tc.tile_pool(name="mbp", bufs=1) as pool_mb,
                            tc.tile_pool(name="attn", bufs=8 * NBO + 8) as pool_attn,
                            tc.tile_pool(name="avst", bufs=8) as pool_avst,
                        ):
                            # K.T and V resident in SBUF for all of P2
                            KT_sb = pool_kvsb.tile([128, CC, T_], BF16, tag="ktsb")
                            for r in range(2):
                                for ss in range(2):
                                    kk = 2 * r + ss
                                    nc.sync.dma_start(
                                        KT_sb[:, :, 512 * kk:512 * kk + 512],
                                        KVs[r, SSB * ss:SSB * (ss + 1)].rearrange(
                                            "(cc p s) -> p cc s", cc=CC, p=128
                                        ),
                                    )
                            V_sb = pool_kvsb.tile([128, NS, C_], BF16, tag="vsb")
                            for r in range(2):
                                nc.scalar.dma_start(
                                    V_sb[:, (NS // 2) * r:(NS // 2) * (r + 1), :],
                                    KVs[r, HKT:2 * HKT].rearrange(
                                        "(ns p d) -> p ns d", ns=NS // 2, p=128
                                    ),
                                )
                            mbt = pool_mb.tile([128, NBO, 4, 256], BF16, tag="mb")
                            nc.scalar.dma_start(mbt[:], mb_d[:].rearrange("nb k p n -> p nb k n"))

                            # owned blocks processed in PAIRS (one K-chunk
                            # stationary load serves two scores matmuls)
                            for grp in range(NBO // 2):
                                js = [2 * grp, 2 * grp + 1]
                                attn = {}
                                for j in js:
                                    attn[j] = [
                                        pool_attn.tile(
                                            [128, 256], BF16, tag="attn", name=f"attn{j}_{k}"
                                        )
                                        for k in range(4 * j + 4)
                                    ]
                                with tc.tile_pool(name="pssc", bufs=6, space="PSUM") as pssc:
                                    for kk in range(2 * grp + 2):
                                        for kl in range(4):
                                            k = 4 * kk + kl
                                            jlist = [j for j in js if 4 * j + 3 >= k]
                                            sps = {
                                                j: pssc.tile(
                                                    [128, 256], F32, tag="sps",
                                                    name=f"sps{grp}_{k}_{j}",
                                                )
                                                for j in jlist
                                            }
                                            for d in range(CC):
                                                for j in jlist:
                                                    nc.tensor.matmul(
                                                        sps[j][:],
                                                        KT_sb[:, d, 128 * k:128 * k + 128],
                                                        QT_sb[:, d, 256 * j:256 * j + 256],
                                                        start=(d == 0),
                                                        stop=(d == CC - 1),
                                                    )
                                            for j in jlist:
                                                if k >= 4 * j:
                                                    nc.vector.tensor_add(
                                                        sps[j][:], sps[j][:],
                                                        mbt[:, j, k - 4 * j, :],
                                                    )
                                                nc.scalar.activation(
                                                    attn[j][k][:], sps[j][:], AF.Exp,
                                                    scale=SCALE,
                                                )
                                for j in js:
                                    n_k = 4 * j + 4
                                    with tc.tile_pool(
                                        name="psav", bufs=8, space="PSUM"
                                    ) as psav:
                                        for d in range(CC):
                                            av = psav.tile(
                                                [128, 256], F32, tag="av",
                                                name=f"av{j}_{d}",
                                            )
                                            for k in range(n_k):
                                                nc.tensor.matmul(
                                                    av[:],
                                                    V_sb[:, k, 128 * d:128 * d + 128],
                                                    attn[j][k][:],
                                                    start=(k == 0),
                                                    stop=(k == n_k - 1),
                                                )
                                            st = pool_avst.tile([128, 256], BF16, tag="st256")
                                            nc.vector.tensor_copy(st[:], av[:])
                                            nc.sync.dma_start(
                                                OT_j[j][:, d, :], st[:]
                                            )
                                with tc.tile_pool(name="psr", bufs=2, space="PSUM") as psr:
                                    for j in js:
                                        for sub in range(2):
                                            rps = psr.tile([128, 2], F32, tag="rps")
                                            for k in range(4 * j + 4):
                                                nc.tensor.matmul(
                                                    rps[:],
                                                    attn[j][k][:, 128 * sub:128 * sub + 128],
                                                    onest[:],
                                                    start=(k == 0),
                                                    stop=(k == 4 * j + 3),
                                                )
                                            nc.vector.reciprocal(
                                                recipt[:, 2 * j + sub:2 * j + sub + 1],
                                                rps[:, 0:1],
                                            )

                        # ======== P3: y = (OT.T @ WoT) * recip ========
                        with (
                            tc.tile_pool(name="wo", bufs=2) as pool_wo,
                            tc.tile_pool(name="ot", bufs=NBO) as pool_ot,
                            tc.tile_pool(name="psf", bufs=6, space="PSUM") as psf,
                        ):
                            otps = [
                                pool_ot.tile([128, CC, 256], BF16, tag=f"ot{j}", name=f"otp{j}")
                                for j in range(NBO)
                            ]
                            for j in range(2):
                                nc.scalar.dma_start(otps[j][:], OT_j[j][:])
                            # wo[0] is a ready input: queue it ahead of the
                            # later-gated OT quarters so it isn't blocked
                            wo0 = pool_wo.tile([128, CC, 512], BF16, tag="wo", name="wo0")
                            nc.scalar.dma_start(wo0[:], WoP_d[0])
                            for j in range(2, NBO):
                                nc.scalar.dma_start(otps[j][:], OT_j[j][:])
                            for e in range(NE):
                                if e == 0:
                                    wo = wo0
                                else:
                                    wo = pool_wo.tile([128, CC, 512], BF16, tag="wo")
                                    nc.scalar.dma_start(wo[:], WoP_d[e])
                                for tsub in range(2 * NBO):
                                    fps = psf.tile([128, 512], F32, tag="fps")
                                    for d in range(CC):
                                        nc.tensor.matmul(
                                            fps[:],
                                            otps[tsub // 2][:, d, 128 * (tsub % 2):128 * (tsub % 2) + 128],
                                            wo[:, d, :],
                                            start=(d == 0),
                                            stop=(d == CC - 1),
                                        )
                                    yt = stage.tile([128, 512], F32, tag="yt")
                                    nc.vector.tensor_scalar_mul(
                                        yt[:], fps[:], recipt[:, tsub:tsub + 1]
                                    )
                                    nc.sync.dma_start(
                                        y_d[128 * tsub:128 * tsub + 128, 512 * e:512 * e + 512],
                                        yt[:],
                                    )
                        pool_cst_cm.__exit__(None, None, None)
    nc.compile()
    return nc


def _host_prep(x, Wq, Wk, Wv, Wo, T_, C_):
    NBO = T_ // 512
    CC = C_ // 128
    NE = C_ // 512
    x = np.asarray(x, np.float32)

    def tile128(W):  # W.T pre-tiled: [d_chunk, p, c_chunk, e128]
        WT = np.asarray(W, np.float32).T.astype(bfloat16)
        return np.ascontiguousarray(WT.reshape(CC, 128, CC, 128).transpose(2, 1, 0, 3))

    def tile512(W):  # W.T pre-tiled: [d_chunk512, p, c_chunk, e512]
        WT = np.asarray(W, np.float32).T.astype(bfloat16)
        return np.ascontiguousarray(WT.reshape(CC, 128, NE, 512).transpose(2, 1, 0, 3))

    WqP = tile128(Wq)
    WkP = tile128(Wk)
    WvP = tile512(Wv)
    WoP = tile512(Wo)
    ones = np.ones((128, 2), bfloat16)
    masks = {}
    own_cols = {}
    for h in range(2):
        mb = np.zeros((NBO, 4, 128, 256), np.float32)
        for p in range(NBO):
            g = 2 * p + h
            t0 = 256 * g
            for kl in range(4):
                s0 = 512 * p + 128 * kl
                s_idx = s0 + np.arange(128)[:, None]
                t_idx = t0 + np.arange(256)[None, :]
                mb[p, kl] = np.where(s_idx <= t_idx, 0.0, NEG)
        masks[h] = mb.astype(bfloat16)
        own_cols[h] = np.concatenate(
            [np.arange(256 * (2 * p + h), 256 * (2 * p + h) + 256) for p in range(NBO)]
        )
    in_maps = []
    for core in range(8):
        b, h = core // 2, core % 2
        xb = x[b % x.shape[0]]
        xT = np.ascontiguousarray(xb.T).astype(bfloat16)
        xTq = np.ascontiguousarray(xT[:, own_cols[h]])
        xTh = np.ascontiguousarray(xT[:, h * (xT.shape[1] // 2):(h + 1) * (xT.shape[1] // 2)])
        in_maps.append(
            {
                "xTh": xTh,
                "xTq": xTq,
                "WqP": WqP,
                "WkP": WkP,
                "WvP": WvP,
                "WoP": WoP,
                "mb": masks[h],
                "ones": ones,
            }
        )
    return in_maps, own_cols


def kernel(x, Wq, Wk, Wv, Wo):
    from concourse.bass_utils import run_bass_kernel_spmd

    T_, C_ = T_FULL, C_FULL
    key = (T_, C_)
    if key not in _CACHE:
        _CACHE[key] = _build(T_, C_)
    nc = _CACHE[key]
    in_maps, own_cols = _host_prep(x, Wq, Wk, Wv, Wo, T_, C_)
    res = run_bass_kernel_spmd(nc, in_maps, list(range(8)))
    NBO = T_ // 512
    y = np.zeros((B, T_, C_), np.float32)
    for core in range(8):
        b, h = core // 2, core % 2
        yc = res.results[core]["y"]
        for p in range(NBO):
            g = 2 * p + h
            y[b, 256 * g:256 * g + 256, :] = yc[256 * p:256 * p + 256, :]
    return y



# revision 6
# speedup vs baseline: 1.1158x; 1.1158x over previous
"""Trainium2 Bass kernel for single-head causal attention (B=4, T=2048, C=2048).

Sharding: 8 cores = 4 batches x 2 t-interleave. Core (b, h) owns the 256-row
blocks {h, 2+h, 4+h, 6+h} of batch b (interleaved for causal load balance).
The two cores of a batch each compute HALF of K.T and V and exchange them in
FOUR piecewise 2-core AllGathers (two K.T d-halves, two V d-halves), each
issued the moment its quarter is produced so the exchange overlaps the rest
of the QKV projections instead of stalling attention. K.T / V land in SBUF
piecewise (loads ride the otherwise-idle GpSimd/Pool queue, behind their
AllGather, so their semaphore waits never block weight/store DMAs).
Attention runs in the "transposed domain" (scores.T = [s, t]) so every
matmul consumes naturally laid-out operands: exp(scale*s + additive mask)
without normalization, softmax denominators via ones-matmul partition
reduction, folded in as a per-partition scale on the final-projection
output, which lands in natural [t, e] layout. Host pre-transposes x /
weights and gathers per-core outputs.

All matmuls run in bf16 (fp32 PSUM accumulation): bf16 weights load via
separate LDWEIGHTS with fast weight load, double-buffered behind the
previous matmul's streaming, so a matmul costs ~N cycles; bf16 also halves
DMA + collective bytes. K.T, V and Q.T stay resident in SBUF for the whole
attention phase. A short warm-up matmul burst at kernel start releases the
PE HAM clock-gate (1.2 -> 2.4 GHz) while the first DMAs are in flight.
"""
import sys

sys.path.insert(0, "/opt/trn_rl_repo")
import numpy as np
from ml_dtypes import bfloat16

_CACHE = {}

B = 4
T_FULL = 2048
C_FULL = 2048
NEG = -1e30


def _build(T_, C_, reps=1):
    import concourse.bacc as bacc
    import concourse.mybir as mybir
    import concourse.tile as tile

    F32 = mybir.dt.float32
    BF16 = mybir.dt.bfloat16
    AF = mybir.ActivationFunctionType
    SCALE = 1.0 / float(np.sqrt(C_FULL))

    CC = C_ // 128      # contraction 128-chunks (also d-chunks)
    NE = C_ // 512      # e-512 chunks for the final projection
    NBO = T_ // 512     # owned 256-blocks per core (j range)
    TOWN = NBO * 256    # owned rows per core
    SK = T_ // 512      # s-512 chunks
    NS = T_ // 128      # s-128 chunks
    NQ = TOWN // 512    # q 512-col chunks
    HCC = CC // 2       # d-chunks per K/V exchange piece

    nc = bacc.Bacc("TRN2", target_bir_lowering=False, debug=False, num_devices=8)
    xTh_d = nc.declare_dram_parameter("xTh", [C_, T_ // 2], BF16, isOutput=False)
    xTq_d = nc.declare_dram_parameter("xTq", [C_, TOWN], BF16, isOutput=False)
    # weights host-pre-tiled to the exact SBUF tile layouts so the loads are
    # contiguous (4KB per partition line vs 256B strided descriptors)
    WqP_d = nc.declare_dram_parameter("WqP", [CC, 128, CC, 128], BF16, isOutput=False)
    WkP_d = nc.declare_dram_parameter("WkP", [CC, 128, CC, 128], BF16, isOutput=False)
    WvP_d = nc.declare_dram_parameter("WvP", [NE, 128, CC, 512], BF16, isOutput=False)
    WoP_d = nc.declare_dram_parameter("WoP", [NE, 128, CC, 512], BF16, isOutput=False)
    mb_d = nc.declare_dram_parameter("mb", [NBO, 4, 128, 256], BF16, isOutput=False)
    ones_d = nc.declare_dram_parameter("ones", [128, 2], BF16, isOutput=False)
    y_d = nc.declare_dram_parameter("y", [TOWN, C_], F32, isOutput=True)

    groups = [[0, 1], [2, 3], [4, 5], [6, 7]]

    with tile.TileContext(nc) as tc:
        with tc.tile_pool(name="dram", bufs=1, space="DRAM") as dram:
            # exchange pieces: [ss|dd', p, chunk', 512] per d-half g
            Kpo = [dram.tile([2, 128, HCC, 512], BF16, tag=f"kpo{g}", name=f"kpo{g}") for g in range(2)]
            Kpa = [dram.tile([2, 2, 128, HCC, 512], BF16, tag=f"kpa{g}", name=f"kpa{g}") for g in range(2)]
            Vpo = [dram.tile([2, 128, HCC, 512], BF16, tag=f"vpo{g}", name=f"vpo{g}") for g in range(2)]
            Vpa = [dram.tile([2, 2, 128, HCC, 512], BF16, tag=f"vpa{g}", name=f"vpa{g}") for g in range(2)]
            # partition-major so P3's reload is one fat contiguous DMA
            OT_j = [dram.tile([128, CC, 256], BF16, tag=f"ot{j}", name=f"otj{j}") for j in range(NBO)]

            with tc.tile_pool(name="stage", bufs=4) as stage:
                # warm the PE (HAM clock-gate releases after ~3.4us of
                # activity) while the first input DMAs are in flight
                with (
                    tc.tile_pool(name="warm", bufs=1) as pool_warm,
                    tc.tile_pool(name="pswm", bufs=1, space="PSUM") as pswm,
                ):
                    wt = pool_warm.tile([128, 512], BF16, tag="warm")
                    nc.vector.memset(wt[:], 0.0)
                    wps = pswm.tile([128, 512], F32, tag="warmps")
                    for _ in range(24):
                        nc.tensor.matmul(
                            wps[:], wt[:, 0:128], wt[:], start=True, stop=True
                        )
                for _rep in range(reps):
                    pool_cst_cm = tc.tile_pool(name="cst", bufs=1)
                    pool_cst = pool_cst_cm.__enter__()
                    onest = pool_cst.tile([128, 2], BF16, tag="ones")
                    nc.scalar.dma_start(onest[:], ones_d[:])
                    recipt = pool_cst.tile([128, 2 * NBO], F32, tag="recip")
                    with tc.tile_pool(name="ktp", bufs=1) as pool_ktp:
                        # K.T resident in SBUF from mid-P1a through attention
                        KT_sb = pool_ktp.tile([128, CC, T_], BF16, tag="ktsb")
                        with tc.tile_pool(name="qsb", bufs=1) as pool_qsb:
                            QT_sb = pool_qsb.tile([128, CC, TOWN], BF16, tag="qtsb")
                            # ==== P1a: K.T = WkT.T @ xT -> Kpo pieces ====
                            with tc.tile_pool(name="xt", bufs=1) as pool_xt:
                                xt = pool_xt.tile([128, CC, T_ // 2], BF16, tag="xt")
                                for c in range(CC):
                                    eng = nc.scalar if c % 2 == 0 else nc.gpsimd
                                    eng.dma_start(
                                        xt[:, c, :], xTh_d[128 * c:128 * c + 128, :]
                                    )
                                with (
                                    tc.tile_pool(name="wk", bufs=2) as pool_w,
                                    tc.tile_pool(name="psk", bufs=8, space="PSUM") as psk,
                                ):
                                    for d in range(CC):
                                        wk = pool_w.tile([128, CC, 128], BF16, tag="wk")
                                        nc.scalar.dma_start(wk[:], WkP_d[d])
                                        kps = [
                                            psk.tile([128, 512], F32, tag="kps", name=f"kps{d}_{ss}")
                                            for ss in range(SK // 2)
                                        ]
                                        for c in range(CC):
                                            for ss in range(SK // 2):
                                                nc.tensor.matmul(
                                                    kps[ss][:],
                                                    wk[:, c, :],
                                                    xt[:, c, 512 * ss:512 * ss + 512],
                                                    start=(c == 0),
                                                    stop=(c == CC - 1),
                                                )
                                        for ss in range(SK // 2):
                                            st = stage.tile([128, 512], BF16, tag="st512")
                                            nc.vector.tensor_copy(st[:], kps[ss][:])
                                            nc.sync.dma_start(
                                                Kpo[d // HCC][ss, :, d % HCC, :], st[:]
                                            )
                                        if d % HCC == HCC - 1:
                                            # piece complete: exchange it now and
                                            # land both ranks' halves in SBUF
                                            g = d // HCC
                                            nc.gpsimd.collective_compute(
                                                "AllGather",
                                                mybir.AluOpType.bypass,
                                                replica_groups=groups,
                                                ins=[Kpo[g][:]],
                                                outs=[Kpa[g][:]],
                                            )
                                            for r in range(2):
                                                for ss in range(2):
                                                    s0 = 1024 * r + 512 * ss
                                                    nc.gpsimd.dma_start(
                                                        KT_sb[:, HCC * g:HCC * g + HCC, s0:s0 + 512],
                                                        Kpa[g][r, ss],
                                                    )
                                # ==== P1b: V = xT.T @ WvT -> Vpo pieces ====
                                with (
                                    tc.tile_pool(name="wv", bufs=2) as pool_wv,
                                    tc.tile_pool(name="psv", bufs=4, space="PSUM") as psv,
                                ):
                                    for dd in range(C_ // 512):
                                        wv = pool_wv.tile([128, CC, 512], BF16, tag="wv")
                                        nc.scalar.dma_start(wv[:], WvP_d[dd])
                                        for s in range(NS // 2):
                                            vps = psv.tile([128, 512], F32, tag="vps")
                                            for c in range(CC):
                                                nc.tensor.matmul(
                                                    vps[:],
                                                    xt[:, c, 128 * s:128 * s + 128],
                                                    wv[:, c, :],
                                                    start=(c == 0),
                                                    stop=(c == CC - 1),
                                                )
                                            st = stage.tile([128, 512], BF16, tag="st512")
                                            nc.vector.tensor_copy(st[:], vps[:])
                                            nc.sync.dma_start(
                                                Vpo[dd // 2][dd % 2, :, s, :], st[:]
                                            )
                                        if dd % 2 == 1:
                                            g = dd // 2
                                            nc.gpsimd.collective_compute(
                                                "AllGather",
                                                mybir.AluOpType.bypass,
                                                replica_groups=groups,
                                                ins=[Vpo[g][:]],
                                                outs=[Vpa[g][:]],
                                            )
                            # V_sb allocated once xt is freed; piece loads ride
                            # the gpsimd queue behind their producing AllGather
                            with tc.tile_pool(name="vp", bufs=1) as pool_vp:
                                V_sb = pool_vp.tile([128, NS, C_], BF16, tag="vsb")
                                for g in range(2):
                                    for r in range(2):
                                        for dd2 in range(2):
                                            d0 = 1024 * g + 512 * dd2
                                            nc.gpsimd.dma_start(
                                                V_sb[:, NS // 2 * r:NS // 2 * r + NS // 2, d0:d0 + 512],
                                                Vpa[g][r, dd2],
                                            )
                                # ==== P1c: Q.T = WqT.T @ xTq -> QT_sb ====
                                with (
                                    tc.tile_pool(name="xtq", bufs=1) as pool_xtq,
                                    tc.tile_pool(name="wq", bufs=2) as pool_wq,
                                    tc.tile_pool(name="psq", bufs=4, space="PSUM") as psq,
                                ):
                                    wq0 = pool_wq.tile([128, CC, 128], BF16, tag="wq", name="wq0")
                                    nc.scalar.dma_start(wq0[:], WqP_d[0])
                                    xtq = pool_xtq.tile([128, CC, TOWN], BF16, tag="xtq")
                                    for c in range(CC):
                                        nc.scalar.dma_start(
                                            xtq[:, c, :], xTq_d[128 * c:128 * c + 128, :]
                                        )
                                    for d in range(CC):
                                        if d == 0:
                                            wq = wq0
                                        else:
                                            wq = pool_wq.tile([128, CC, 128], BF16, tag="wq")
                                            nc.scalar.dma_start(wq[:], WqP_d[d])
                                        for tt in range(NQ):
                                            qps = psq.tile([128, 512], F32, tag="qps")
                                            for c in range(CC):
                                                nc.tensor.matmul(
                                                    qps[:],
                                                    wq[:, c, :],
                                                    xtq[:, c, 512 * tt:512 * tt + 512],
                                                    start=(c == 0),
                                                    stop=(c == CC - 1),
                                                )
                                            nc.vector.tensor_copy(
                                                QT_sb[:, d, 512 * tt:512 * tt + 512], qps[:]
                                            )

                                # ==== P2: attention per owned block j ====
                                with (
                                    tc.tile_pool(name="mbp", bufs=1) as pool_mb,
                                    tc.tile_pool(name="attn", bufs=8 * NBO + 8) as pool_attn,
                                    tc.tile_pool(name="avst", bufs=8) as pool_avst,
                                ):
                                    mbt = pool_mb.tile([128, NBO, 4, 256], BF16, tag="mb")
                                    nc.scalar.dma_start(mbt[:], mb_d[:].rearrange("nb k p n -> p nb k n"))

                                    # owned blocks processed in PAIRS (one K-chunk
                                    # stationary load serves two scores matmuls)
                                    for grp in range(NBO // 2):
                                        js = [2 * grp, 2 * grp + 1]
                                        attn = {}
                                        for j in js:
                                            attn[j] = [
                                                pool_attn.tile(
                                                    [128, 256], BF16, tag="attn", name=f"attn{j}_{k}"
                                                )
                                                for k in range(4 * j + 4)
                                            ]
                                        with tc.tile_pool(name="pssc", bufs=6, space="PSUM") as pssc:
                                            for kk in range(2 * grp + 2):
                                                for kl in range(4):
                                                    k = 4 * kk + kl
                                                    jlist = [j for j in js if 4 * j + 3 >= k]
                                                    sps = {
                                                        j: pssc.tile(
                                                            [128, 256], F32, tag="sps",
                                                            name=f"sps{grp}_{k}_{j}",
                                                        )
                                                        for j in jlist
                                                    }
                                                    for d in range(CC):
                                                        for j in jlist:
                                                            nc.tensor.matmul(
                                                                sps[j][:],
                                                                KT_sb[:, d, 128 * k:128 * k + 128],
                                                                QT_sb[:, d, 256 * j:256 * j + 256],
                                                                start=(d == 0),
                                                                stop=(d == CC - 1),
                                                            )
                                                    for j in jlist:
                                                        if k >= 4 * j:
                                                            nc.vector.tensor_add(
                                                                sps[j][:], sps[j][:],
                                                                mbt[:, j, k - 4 * j, :],
                                                            )
                                                        nc.scalar.activation(
                                                            attn[j][k][:], sps[j][:], AF.Exp,
                                                            scale=SCALE,
                                                        )
                                        for j in js:
                                            n_k = 4 * j + 4
                                            with tc.tile_pool(
                                                name="psav", bufs=8, space="PSUM"
                                            ) as psav:
                                                for d in range(CC):
                                                    av = psav.tile(
                                                        [128, 256], F32, tag="av",
                                                        name=f"av{j}_{d}",
                                                    )
                                                    for k in range(n_k):
                                                        nc.tensor.matmul(
                                                            av[:],
                                                            V_sb[:, k, 128 * d:128 * d + 128],
                                                            attn[j][k][:],
                                                            start=(k == 0),
                                                            stop=(k == n_k - 1),
                                                        )
                                                    st = pool_avst.tile([128, 256], BF16, tag="st256")
                                                    nc.vector.tensor_copy(st[:], av[:])
                                                    nc.sync.dma_start(
                                                        OT_j[j][:, d, :], st[:]
                                                    )
                                        with tc.tile_pool(name="psr", bufs=2, space="PSUM") as psr:
                                            for j in js:
                                                for sub in range(2):
                                                    rps = psr.tile([128, 2], F32, tag="rps")
                                                    for k in range(4 * j + 4):
                                                        nc.tensor.matmul(
                                                            rps[:],
                                                            attn[j][k][:, 128 * sub:128 * sub + 128],
                                                            onest[:],
                                                            start=(k == 0),
                                                            stop=(k == 4 * j + 3),
                                                        )
                                                    nc.vector.reciprocal(
                                                        recipt[:, 2 * j + sub:2 * j + sub + 1],
                                                        rps[:, 0:1],
                                                    )

                    # ======== P3: y = (OT.T @ WoT) * recip ========
                    with (
                        tc.tile_pool(name="wo", bufs=2) as pool_wo,
                        tc.tile_pool(name="ot", bufs=NBO) as pool_ot,
                        tc.tile_pool(name="yst", bufs=4) as pool_yst,
                        tc.tile_pool(name="psf", bufs=6, space="PSUM") as psf,
                    ):
                        otps = [
                            pool_ot.tile([128, CC, 256], BF16, tag=f"ot{j}", name=f"otp{j}")
                            for j in range(NBO)
                        ]
                        for j in range(2):
                            nc.scalar.dma_start(otps[j][:], OT_j[j][:])
                        # wo[0] is a ready input: queue it ahead of the
                        # later-gated OT quarters so it isn't blocked
                        wo0 = pool_wo.tile([128, CC, 512], BF16, tag="wo", name="wo0")
                        nc.scalar.dma_start(wo0[:], WoP_d[0])
                        for j in range(2, NBO):
                            nc.scalar.dma_start(otps[j][:], OT_j[j][:])
                        for e in range(NE):
                            if e == 0:
                                wo = wo0
                            else:
                                wo = pool_wo.tile([128, CC, 512], BF16, tag="wo")
                                nc.scalar.dma_start(wo[:], WoP_d[e])
                            for tsub in range(2 * NBO):
                                fps = psf.tile([128, 512], F32, tag="fps")
                                for d in range(CC):
                                    nc.tensor.matmul(
                                        fps[:],
                                        otps[tsub // 2][:, d, 128 * (tsub % 2):128 * (tsub % 2) + 128],
                                        wo[:, d, :],
                                        start=(d == 0),
                                        stop=(d == CC - 1),
                                    )
                                yt = pool_yst.tile([128, 512], F32, tag="yt")
                                nc.vector.tensor_scalar_mul(
                                    yt[:], fps[:], recipt[:, tsub:tsub + 1]
                                )
                                nc.sync.dma_start(
                                    y_d[128 * tsub:128 * tsub + 128, 512 * e:512 * e + 512],
                                    yt[:],
                                )
                    pool_cst_cm.__exit__(None, None, None)
    nc.compile()
    return nc


def _host_prep(x, Wq, Wk, Wv, Wo, T_, C_):
    NBO = T_ // 512
    CC = C_ // 128
    NE = C_ // 512
    x = np.asarray(x, np.float32)

    def tile128(W):  # W.T pre-tiled: [d_chunk, p, c_chunk, e128]
        WT = np.asarray(W, np.float32).T.astype(bfloat16)
        return np.ascontiguousarray(WT.reshape(CC, 128, CC, 128).transpose(2, 1, 0, 3))

    def tile512(W):  # W.T pre-tiled: [d_chunk512, p, c_chunk, e512]
        WT = np.asarray(W, np.float32).T.astype(bfloat16)
        return np.ascontiguousarray(WT.reshape(CC, 128, NE, 512).transpose(2, 1, 0, 3))

    WqP = tile128(Wq)
    WkP = tile128(Wk)
    WvP = tile512(Wv)
    WoP = tile512(Wo)
    ones = np.ones((128, 2), bfloat16)
    masks = {}
    own_cols = {}
    for h in range(2):
        mb = np.zeros((NBO, 4, 128, 256), np.float32)
        for p in range(NBO):
            g = 2 * p + h
            t0 = 256 * g
            for kl in range(4):
                s0 = 512 * p + 128 * kl
                s_idx = s0 + np.arange(128)[:, None]
                t_idx = t0 + np.arange(256)[None, :]
                mb[p, kl] = np.where(s_idx <= t_idx, 0.0, NEG)
        masks[h] = mb.astype(bfloat16)
        own_cols[h] = np.concatenate(
            [np.arange(256 * (2 * p + h), 256 * (2 * p + h) + 256) for p in range(NBO)]
        )
    in_maps = []
    for core in range(8):
        b, h = core // 2, core % 2
        xb = x[b % x.shape[0]]
        xT = np.ascontiguousarray(xb.T).astype(bfloat16)
        xTq = np.ascontiguousarray(xT[:, own_cols[h]])
        xTh = np.ascontiguousarray(xT[:, h * (xT.shape[1] // 2):(h + 1) * (xT.shape[1] // 2)])
        in_maps.append(
            {
                "xTh": xTh,
                "xTq": xTq,
                "WqP": WqP,
                "WkP": WkP,
                "WvP": WvP,
                "WoP": WoP,
                "mb": masks[h],
                "ones": ones,
            }
        )
    return in_maps, own_cols


def kernel(x, Wq, Wk, Wv, Wo):
    from concourse.bass_utils import run_bass_kernel_spmd

    T_, C_ = T_FULL, C_FULL
    key = (T_, C_)
    if key not in _CACHE:
        _CACHE[key] = _build(T_, C_)
    nc = _CACHE[key]
    in_maps, own_cols = _host_prep(x, Wq, Wk, Wv, Wo, T_, C_)
    res = run_bass_kernel_spmd(nc, in_maps, list(range(8)))
    NBO = T_ // 512
    y = np.zeros((B, T_, C_), np.float32)
    for core in range(8):
        b, h = core // 2, core % 2
        yc = res.results[core]["y"]
        for p in range(NBO):
            g = 2 * p + h
            y[b, 256 * g:256 * g + 256, :] = yc[256 * p:256 * p + 256, :]
    return y


# revision 9
# speedup vs baseline: 1.1748x; 1.0529x over previous
"""Trainium2 Bass kernel for single-head causal attention (B=4, T=2048, C=2048).

Sharding: 8 cores = 4 batches x 2 t-interleave. Core (b, h) owns the 256-row
blocks {h, 2+h, 4+h, 6+h} of batch b (interleaved for causal load balance).
The two cores of a batch each compute HALF of K.T and V and exchange them in
FOUR piecewise 2-core AllGathers (two K.T d-halves, two V d-halves), each
issued the moment its quarter is produced so the exchange overlaps the rest
of the QKV projections instead of stalling attention. K.T / V land in SBUF
piecewise (loads ride the otherwise-idle GpSimd/Pool queue, behind their
AllGather, so their semaphore waits never block weight/store DMAs).
Attention runs in the "transposed domain" (scores.T = [s, t]) so every
matmul consumes naturally laid-out operands: exp(scale*s + additive mask)
without normalization, softmax denominators via ones-matmul partition
reduction, folded in as a per-partition scale on the final-projection
output, which lands in natural [t, e] layout. Host pre-transposes x /
weights and gathers per-core outputs.

All matmuls run in bf16 (fp32 PSUM accumulation): bf16 weights load via
separate LDWEIGHTS with fast weight load, double-buffered behind the
previous matmul's streaming, so a matmul costs ~N cycles; bf16 also halves
DMA + collective bytes. K.T, V and Q.T stay resident in SBUF for the whole
attention phase. A short warm-up matmul burst at kernel start releases the
PE HAM clock-gate (1.2 -> 2.4 GHz) while the first DMAs are in flight.
"""
import sys

sys.path.insert(0, "/opt/trn_rl_repo")
import numpy as np
from ml_dtypes import bfloat16

_CACHE = {}

B = 4
T_FULL = 2048
C_FULL = 2048
NEG = -1e30


def _build(T_, C_, reps=1):
    import concourse.bacc as bacc
    import concourse.mybir as mybir
    import concourse.tile as tile

    F32 = mybir.dt.float32
    BF16 = mybir.dt.bfloat16
    AF = mybir.ActivationFunctionType
    SCALE = 1.0 / float(np.sqrt(C_FULL))

    CC = C_ // 128      # contraction 128-chunks (also d-chunks)
    NE = C_ // 512      # e-512 chunks for the final projection
    NBO = T_ // 512     # owned 256-blocks per core (j range)
    TOWN = NBO * 256    # owned rows per core
    SK = T_ // 512      # s-512 chunks
    NS = T_ // 128      # s-128 chunks
    NQ = TOWN // 512    # q 512-col chunks
    HCC = CC // 2       # d-chunks per K/V exchange piece

    nc = bacc.Bacc("TRN2", target_bir_lowering=False, debug=False, num_devices=8)
    xTh_d = nc.declare_dram_parameter("xTh", [C_, T_ // 2], BF16, isOutput=False)
    xTq_d = nc.declare_dram_parameter("xTq", [C_, TOWN], BF16, isOutput=False)
    # weights host-pre-tiled to the exact SBUF tile layouts so the loads are
    # contiguous (4KB per partition line vs 256B strided descriptors)
    WqP_d = nc.declare_dram_parameter("WqP", [CC, 128, CC, 128], BF16, isOutput=False)
    WkP_d = nc.declare_dram_parameter("WkP", [CC, 128, CC, 128], BF16, isOutput=False)
    WvP_d = nc.declare_dram_parameter("WvP", [NE, 128, CC, 512], BF16, isOutput=False)
    WoP_d = nc.declare_dram_parameter("WoP", [NE, 128, CC, 512], BF16, isOutput=False)
    mb_d = nc.declare_dram_parameter("mb", [NBO, 4, 128, 256], BF16, isOutput=False)
    ones_d = nc.declare_dram_parameter("ones", [128, 2], BF16, isOutput=False)
    y_d = nc.declare_dram_parameter("y", [TOWN, C_], F32, isOutput=True)

    groups = [[0, 1], [2, 3], [4, 5], [6, 7]]

    with tile.TileContext(nc) as tc:
        with tc.tile_pool(name="dram", bufs=1, space="DRAM") as dram:
            # exchange pieces: [ss|dd', p, chunk', 512] per d-half g
            Kpo = [dram.tile([2, 128, HCC, 512], BF16, tag=f"kpo{g}", name=f"kpo{g}") for g in range(2)]
            Kpa = [dram.tile([2, 2, 128, HCC, 512], BF16, tag=f"kpa{g}", name=f"kpa{g}") for g in range(2)]
            Vpo = [dram.tile([128, HCC, 512], BF16, tag=f"vpo{g}", name=f"vpo{g}") for g in range(NE)]
            Vpa = [dram.tile([2, 128, HCC, 512], BF16, tag=f"vpa{g}", name=f"vpa{g}") for g in range(NE)]
            # partition-major so P3's reload is one fat contiguous DMA
            OT_j = [dram.tile([128, CC, 256], BF16, tag=f"ot{j}", name=f"otj{j}") for j in range(NBO)]

            with tc.tile_pool(name="stage", bufs=4) as stage:
                # warm the PE (HAM clock-gate releases after ~3.4us of
                # activity) while the first input DMAs are in flight
                with (
                    tc.tile_pool(name="warm", bufs=1) as pool_warm,
                    tc.tile_pool(name="pswm", bufs=1, space="PSUM") as pswm,
                ):
                    wt = pool_warm.tile([128, 512], BF16, tag="warm")
                    nc.vector.memset(wt[:], 0.0)
                    wps = pswm.tile([128, 512], F32, tag="warmps")
                    for _ in range(24):
                        nc.tensor.matmul(
                            wps[:], wt[:, 0:128], wt[:], start=True, stop=True
                        )
                for _rep in range(reps):
                    pool_cst_cm = tc.tile_pool(name="cst", bufs=1)
                    pool_cst = pool_cst_cm.__enter__()
                    onest = pool_cst.tile([128, 2], BF16, tag="ones")
                    nc.scalar.dma_start(onest[:], ones_d[:])
                    recipt = pool_cst.tile([128, 2 * NBO], F32, tag="recip")
                    with tc.tile_pool(name="ktp", bufs=1) as pool_ktp:
                        # K.T resident in SBUF from mid-P1a through attention
                        KT_sb = pool_ktp.tile([128, CC, T_], BF16, tag="ktsb")
                        with tc.tile_pool(name="qsb", bufs=1) as pool_qsb:
                            QT_sb = pool_qsb.tile([128, CC, TOWN], BF16, tag="qtsb")
                            # ==== P1a: K.T = WkT.T @ xT -> Kpo pieces ====
                            with tc.tile_pool(name="xt", bufs=1) as pool_xt:
                                xt = pool_xt.tile([128, CC, T_ // 2], BF16, tag="xt")
                                for c in range(CC):
                                    eng = nc.scalar if c % 2 == 0 else nc.gpsimd
                                    eng.dma_start(
                                        xt[:, c, :], xTh_d[128 * c:128 * c + 128, :]
                                    )
                                with (
                                    tc.tile_pool(name="wk", bufs=2) as pool_w,
                                    tc.tile_pool(name="psk", bufs=8, space="PSUM") as psk,
                                ):
                                    for d in range(CC):
                                        wk = pool_w.tile([128, CC, 128], BF16, tag="wk")
                                        nc.scalar.dma_start(wk[:], WkP_d[d])
                                        kps = [
                                            psk.tile([128, 512], F32, tag="kps", name=f"kps{d}_{ss}")
                                            for ss in range(SK // 2)
                                        ]
                                        for c in range(CC):
                                            for ss in range(SK // 2):
                                                nc.tensor.matmul(
                                                    kps[ss][:],
                                                    wk[:, c, :],
                                                    xt[:, c, 512 * ss:512 * ss + 512],
                                                    start=(c == 0),
                                                    stop=(c == CC - 1),
                                                )
                                        for ss in range(SK // 2):
                                            st = stage.tile([128, 512], BF16, tag="st512")
                                            nc.vector.tensor_copy(st[:], kps[ss][:])
                                            nc.sync.dma_start(
                                                Kpo[d // HCC][ss, :, d % HCC, :], st[:]
                                            )
                                        if d % HCC == HCC - 1:
                                            # piece complete: exchange it now and
                                            # land both ranks' halves in SBUF
                                            g = d // HCC
                                            nc.gpsimd.collective_compute(
                                                "AllGather",
                                                mybir.AluOpType.bypass,
                                                replica_groups=groups,
                                                ins=[Kpo[g][:]],
                                                outs=[Kpa[g][:]],
                                            )
                                            for r in range(2):
                                                for ss in range(2):
                                                    s0 = 1024 * r + 512 * ss
                                                    nc.gpsimd.dma_start(
                                                        KT_sb[:, HCC * g:HCC * g + HCC, s0:s0 + 512],
                                                        Kpa[g][r, ss],
                                                    )
                                # ==== P1b: V = xT.T @ WvT -> Vpo pieces ====
                                with (
                                    tc.tile_pool(name="wv", bufs=2) as pool_wv,
                                    tc.tile_pool(name="psv", bufs=4, space="PSUM") as psv,
                                ):
                                    for dd in range(C_ // 512):
                                        wv = pool_wv.tile([128, CC, 512], BF16, tag="wv")
                                        nc.scalar.dma_start(wv[:], WvP_d[dd])
                                        for s in range(NS // 2):
                                            vps = psv.tile([128, 512], F32, tag="vps")
                                            for c in range(CC):
                                                nc.tensor.matmul(
                                                    vps[:],
                                                    xt[:, c, 128 * s:128 * s + 128],
                                                    wv[:, c, :],
                                                    start=(c == 0),
                                                    stop=(c == CC - 1),
                                                )
                                            st = stage.tile([128, 512], BF16, tag="st512")
                                            nc.vector.tensor_copy(st[:], vps[:])
                                            nc.sync.dma_start(
                                                Vpo[dd][:, s, :], st[:]
                                            )
                                        nc.gpsimd.collective_compute(
                                            "AllGather",
                                            mybir.AluOpType.bypass,
                                            replica_groups=groups,
                                            ins=[Vpo[dd][:]],
                                            outs=[Vpa[dd][:]],
                                        )
                            # V_sb allocated once xt is freed; piece loads ride
                            # the gpsimd queue behind their producing AllGather
                            with tc.tile_pool(name="vp", bufs=1) as pool_vp:
                                V_sb = pool_vp.tile([128, NS, C_], BF16, tag="vsb")
                                for dd in range(NE):
                                    for r in range(2):
                                        d0 = 512 * dd
                                        nc.gpsimd.dma_start(
                                            V_sb[:, NS // 2 * r:NS // 2 * r + NS // 2, d0:d0 + 512],
                                            Vpa[dd][r],
                                        )
                                # ==== P1c: Q.T = WqT.T @ xTq -> QT_sb ====
                                with (
                                    tc.tile_pool(name="xtq", bufs=1) as pool_xtq,
                                    tc.tile_pool(name="wq", bufs=2) as pool_wq,
                                    tc.tile_pool(name="psq", bufs=4, space="PSUM") as psq,
                                ):
                                    wq0 = pool_wq.tile([128, CC, 128], BF16, tag="wq", name="wq0")
                                    nc.scalar.dma_start(wq0[:], WqP_d[0])
                                    xtq = pool_xtq.tile([128, CC, TOWN], BF16, tag="xtq")
                                    for c in range(CC):
                                        nc.scalar.dma_start(
                                            xtq[:, c, :], xTq_d[128 * c:128 * c + 128, :]
                                        )
                                    for d in range(CC):
                                        if d == 0:
                                            wq = wq0
                                        else:
                                            wq = pool_wq.tile([128, CC, 128], BF16, tag="wq")
                                            nc.scalar.dma_start(wq[:], WqP_d[d])
                                        for tt in range(NQ):
                                            qps = psq.tile([128, 512], F32, tag="qps")
                                            for c in range(CC):
                                                nc.tensor.matmul(
                                                    qps[:],
                                                    wq[:, c, :],
                                                    xtq[:, c, 512 * tt:512 * tt + 512],
                                                    start=(c == 0),
                                                    stop=(c == CC - 1),
                                                )
                                            nc.vector.tensor_copy(
                                                QT_sb[:, d, 512 * tt:512 * tt + 512], qps[:]
                                            )

                                # ==== P2: attention per owned block j ====
                                with (
                                    tc.tile_pool(name="mbp", bufs=1) as pool_mb,
                                    tc.tile_pool(name="attn", bufs=8 * NBO + 8) as pool_attn,
                                    tc.tile_pool(name="avst", bufs=8) as pool_avst,
                                ):
                                    mbt = pool_mb.tile([128, NBO, 4, 256], BF16, tag="mb")
                                    nc.scalar.dma_start(mbt[:], mb_d[:].rearrange("nb k p n -> p nb k n"))

                                    # owned blocks processed in PAIRS (one K-chunk
                                    # stationary load serves two scores matmuls)
                                    for grp in range(NBO // 2):
                                        js = [2 * grp, 2 * grp + 1]
                                        attn = {}
                                        for j in js:
                                            attn[j] = [
                                                pool_attn.tile(
                                                    [128, 256], BF16, tag="attn", name=f"attn{j}_{k}"
                                                )
                                                for k in range(4 * j + 4)
                                            ]
                                        with tc.tile_pool(name="pssc", bufs=6, space="PSUM") as pssc:
                                            for kk in range(2 * grp + 2):
                                                for kl in range(4):
                                                    k = 4 * kk + kl
                                                    jlist = [j for j in js if 4 * j + 3 >= k]
                                                    sps = {
                                                        j: pssc.tile(
                                                            [128, 256], F32, tag="sps",
                                                            name=f"sps{grp}_{k}_{j}",
                                                        )
                                                        for j in jlist
                                                    }
                                                    for d in range(CC):
                                                        for j in jlist:
                                                            nc.tensor.matmul(
                                                                sps[j][:],
                                                                KT_sb[:, d, 128 * k:128 * k + 128],
                                                                QT_sb[:, d, 256 * j:256 * j + 256],
                                                                start=(d == 0),
                                                                stop=(d == CC - 1),
                                                            )
                                                    for j in jlist:
                                                        if k >= 4 * j:
                                                            nc.vector.tensor_add(
                                                                sps[j][:], sps[j][:],
                                                                mbt[:, j, k - 4 * j, :],
                                                            )
                                                        nc.scalar.activation(
                                                            attn[j][k][:], sps[j][:], AF.Exp,
                                                            scale=SCALE,
                                                        )
                                        for j in js:
                                            n_k = 4 * j + 4
                                            with tc.tile_pool(
                                                name="psav", bufs=8, space="PSUM"
                                            ) as psav:
                                                for d in range(CC):
                                                    av = psav.tile(
                                                        [128, 256], F32, tag="av",
                                                        name=f"av{j}_{d}",
                                                    )
                                                    for k in range(n_k):
                                                        nc.tensor.matmul(
                                                            av[:],
                                                            V_sb[:, k, 128 * d:128 * d + 128],
                                                            attn[j][k][:],
                                                            start=(k == 0),
                                                            stop=(k == n_k - 1),
                                                        )
                                                    st = pool_avst.tile([128, 256], BF16, tag="st256")
                                                    nc.vector.tensor_copy(st[:], av[:])
                                                    nc.sync.dma_start(
                                                        OT_j[j][:, d, :], st[:]
                                                    )
                                        with tc.tile_pool(name="psr", bufs=2, space="PSUM") as psr:
                                            for j in js:
                                                for sub in range(2):
                                                    rps = psr.tile([128, 2], F32, tag="rps")
                                                    for k in range(4 * j + 4):
                                                        nc.tensor.matmul(
                                                            rps[:],
                                                            attn[j][k][:, 128 * sub:128 * sub + 128],
                                                            onest[:],
                                                            start=(k == 0),
                                                            stop=(k == 4 * j + 3),
                                                        )
                                                    nc.vector.reciprocal(
                                                        recipt[:, 2 * j + sub:2 * j + sub + 1],
                                                        rps[:, 0:1],
                                                    )

                    # ======== P3: y = (OT.T @ WoT) * recip ========
                    with (
                        tc.tile_pool(name="wo", bufs=2) as pool_wo,
                        tc.tile_pool(name="ot", bufs=NBO) as pool_ot,
                        tc.tile_pool(name="yst", bufs=4) as pool_yst,
                        tc.tile_pool(name="psf", bufs=6, space="PSUM") as psf,
                    ):
                        otps = [
                            pool_ot.tile([128, CC, 256], BF16, tag=f"ot{j}", name=f"otp{j}")
                            for j in range(NBO)
                        ]
                        for j in range(2):
                            nc.scalar.dma_start(otps[j][:], OT_j[j][:])
                        # wo[0] is a ready input: queue it ahead of the
                        # later-gated OT quarters so it isn't blocked
                        wo0 = pool_wo.tile([128, CC, 512], BF16, tag="wo", name="wo0")
                        nc.scalar.dma_start(wo0[:], WoP_d[0])
                        for j in range(2, NBO):
                            nc.scalar.dma_start(otps[j][:], OT_j[j][:])
                        for e in range(NE):
                            if e == 0:
                                wo = wo0
                            else:
                                wo = pool_wo.tile([128, CC, 512], BF16, tag="wo")
                                nc.scalar.dma_start(wo[:], WoP_d[e])
                            for tsub in range(2 * NBO):
                                fps = psf.tile([128, 512], F32, tag="fps")
                                for d in range(CC):
                                    nc.tensor.matmul(
                                        fps[:],
                                        otps[tsub // 2][:, d, 128 * (tsub % 2):128 * (tsub % 2) + 128],
                                        wo[:, d, :],
                                        start=(d == 0),
                                        stop=(d == CC - 1),
                                    )
                                yt = pool_yst.tile([128, 512], F32, tag="yt")
                                nc.vector.tensor_scalar_mul(
                                    yt[:], fps[:], recipt[:, tsub:tsub + 1]
                                )
                                nc.sync.dma_start(
                                    y_d[128 * tsub:128 * tsub + 128, 512 * e:512 * e + 512],
                                    yt[:],
                                )
                    pool_cst_cm.__exit__(None, None, None)
    nc.compile()
    return nc


def _host_prep(x, Wq, Wk, Wv, Wo, T_, C_):
    NBO = T_ // 512
    CC = C_ // 128
    NE = C_ // 512
    x = np.asarray(x, np.float32)

    def tile128(W):  # W.T pre-tiled: [d_chunk, p, c_chunk, e128]
        WT = np.asarray(W, np.float32).T.astype(bfloat16)
        return np.ascontiguousarray(WT.reshape(CC, 128, CC, 128).transpose(2, 1, 0, 3))

    def tile512(W):  # W.T pre-tiled: [d_chunk512, p, c_chunk, e512]
        WT = np.asarray(W, np.float32).T.astype(bfloat16)
        return np.ascontiguousarray(WT.reshape(CC, 128, NE, 512).transpose(2, 1, 0, 3))

    WqP = tile128(Wq)
    WkP = tile128(Wk)
    WvP = tile512(Wv)
    WoP = tile512(Wo)
    ones = np.ones((128, 2), bfloat16)
    masks = {}
    own_cols = {}
    for h in range(2):
        mb = np.zeros((NBO, 4, 128, 256), np.float32)
        for p in range(NBO):
            g = 2 * p + h
            t0 = 256 * g
            for kl in range(4):
                s0 = 512 * p + 128 * kl
                s_idx = s0 + np.arange(128)[:, None]
                t_idx = t0 + np.arange(256)[None, :]
                mb[p, kl] = np.where(s_idx <= t_idx, 0.0, NEG)
        masks[h] = mb.astype(bfloat16)
        own_cols[h] = np.concatenate(
            [np.arange(256 * (2 * p + h), 256 * (2 * p + h) + 256) for p in range(NBO)]
        )
    in_maps = []
    for core in range(8):
        b, h = core // 2, core % 2
        xb = x[b % x.shape[0]]
        xT = np.ascontiguousarray(xb.T).astype(bfloat16)
        xTq = np.ascontiguousarray(xT[:, own_cols[h]])
        xTh = np.ascontiguousarray(xT[:, h * (xT.shape[1] // 2):(h + 1) * (xT.shape[1] // 2)])
        in_maps.append(
            {
                "xTh": xTh,
                "xTq": xTq,
                "WqP": WqP,
                "WkP": WkP,
                "WvP": WvP,
                "WoP": WoP,
                "mb": masks[h],
                "ones": ones,
            }
        )
    return in_maps, own_cols


def kernel(x, Wq, Wk, Wv, Wo):
    from concourse.bass_utils import run_bass_kernel_spmd

    T_, C_ = T_FULL, C_FULL
    key = (T_, C_)
    if key not in _CACHE:
        _CACHE[key] = _build(T_, C_)
    nc = _CACHE[key]
    in_maps, own_cols = _host_prep(x, Wq, Wk, Wv, Wo, T_, C_)
    res = run_bass_kernel_spmd(nc, in_maps, list(range(8)))
    NBO = T_ // 512
    y = np.zeros((B, T_, C_), np.float32)
    for core in range(8):
        b, h = core // 2, core % 2
        yc = res.results[core]["y"]
        for p in range(NBO):
            g = 2 * p + h
            y[b, 256 * g:256 * g + 256, :] = yc[256 * p:256 * p + 256, :]
    return y
